# revision 25
# baseline (speedup 1.0000x reference)
"""Trainium2 Bass kernel for causal multi-head attention with RoPE.

Problem: B=2, S=2048, DIM=1024, 16 heads, head_dim=64.
  q = x @ Wq.T ; k = x @ Wk.T ; v = x @ Wv.T        (torch Linear convention)
  q, k = rope(q), rope(k)                            (Llama interleaved pairs)
  y = softmax(causal(q k^T / 8)) v @ Wo.T

Sharding (8 cores): data-parallel over batch (2) x tensor-parallel over
head groups (4 heads per core).  Wq/Wk/Wv row-sharded, Wo column-sharded;
the host sums the 4 partial outputs per batch.

v3 schedule: the attention phase is ACT(exp)-bound (~105us floor: 69.6k
exp columns + ~0.5us/instruction pipeline refill, 96 instructions).  So
everything else is arranged to hide UNDER that stream:

  - head-serial attention units (one head per unit, 8 units) shrink the
    PSUM obs accumulator to 2 banks: stp 4 + obs 2 + aux 2 = 8 banks.
  - ALL projections run as 1-bank "quarter passes" through the 2-bank aux
    pool, so they can execute concurrently with attention.
  - a trickle queue of small PE work items (V projection, second-chunk
    Q/K projections, first-half output projection) is pumped one item per
    score-block inside the attention j-loop, filling the PE slack that
    the exp stream leaves (~0.2-0.5us per block).
  - prefix before the first exp is just the pair-0 Q/K projections;
    PV matmuls tolerate V-projection lag (obs accumulation order is
    free), so V trickles from t=0 without stalling the exp stream.
  - unit order pair-major ([p0: qt0 h0,h1, qt1 h0,h1], [p1: ...]) so the
    pair-1 projections have 4 units of slack to trickle through, and the
    tokens[0:1024) output projection unlocks after unit 6.
"""

import os
import sys

sys.path.insert(0, "/opt/trn_rl_repo")

import numpy as np

import concourse.bass as bass
import concourse.mybir as mybir
import concourse.tile as tile
from concourse import bacc
from concourse.bass_utils import run_bass_kernel_spmd

F16 = mybir.dt.float16
F32 = mybir.dt.float32

DIM = 1024
NUM_HEADS = 16
HEAD_DIM = 64
B = 2
GROUPS = 4                   # head groups (tensor parallel)
HPG = NUM_HEADS // GROUPS    # heads per group = 4
FG = HPG * HEAD_DIM          # features per group = 256
THETA = 10000.0


def build_program(S=2048):
    from collections import deque
    from contextlib import ExitStack

    nc = bacc.Bacc(None, target_bir_lowering=False)
    NT = S // 128                 # token blocks
    QTILE = min(1024, S)
    NQT = S // QTILE
    HALF = S // 2

    xt_d = nc.declare_dram_parameter("xt", [DIM, S], F16, isOutput=False)
    wq_d = nc.declare_dram_parameter("wqt", [DIM, FG], F16, isOutput=False)
    wk_d = nc.declare_dram_parameter("wkt", [DIM, FG], F16, isOutput=False)
    wv_d = nc.declare_dram_parameter("wvt", [DIM, FG], F16, isOutput=False)
    wo_d = nc.declare_dram_parameter("wot", [FG, DIM], F16, isOutput=False)
    cos_d = nc.declare_dram_parameter("cos", [128, S], F16, isOutput=False)
    sin_d = nc.declare_dram_parameter("sins", [128, S], F16, isOutput=False)
    mask_d = nc.declare_dram_parameter("mask", [128, 128], F16, isOutput=False)
    # fp16 partial output; host upcasts to fp32 before summing the 4 partials
    yt_d = nc.declare_dram_parameter("yt", [DIM, S], F16, isOutput=True)

    Exp = mybir.ActivationFunctionType.Exp

    with tile.TileContext(nc) as tc:
        with ExitStack() as ctx:
            consts = ctx.enter_context(tc.tile_pool(name="consts", bufs=1))

            qt_sb = consts.tile([128, 2, S], F16)
            kt_sb = consts.tile([128, 2, S], F16)
            vaug = consts.tile([128, NT, HPG * 65], F16)
            zt_sb = consts.tile([128, 2, S], F16)

            # warm the exp table on ACT immediately (the ~2.7us
            # ACT_TABLE_LOAD overlaps the input DMA)
            warm = consts.tile([1, 8], F32)
            nc.vector.memset(warm[:], 0.0)
            warm2 = consts.tile([1, 8], F16)
            nc.scalar.activation(warm2[:], warm[:], Exp)

            wpool = ctx.enter_context(tc.tile_pool(name="wpool", bufs=1))
            wq_sb = wpool.tile([128, 8, FG], F16)
            nc.sync.dma_start(wq_sb[:], wq_d[:].rearrange("(c p) f -> p c f", p=128))
            wk_sb = wpool.tile([128, 8, FG], F16)
            nc.sync.dma_start(wk_sb[:], wk_d[:].rearrange("(c p) f -> p c f", p=128))
            xt_sb = wpool.tile([128, 8, S], F16)
            xt_r = xt_d[:].rearrange("(c p) t -> p c t", p=128)
            for k in range(8):
                nc.sync.dma_start(xt_sb[:, k, 0:HALF], xt_r[:, k, 0:HALF])
            cos_t = consts.tile([128, S], F16)
            nc.sync.dma_start(cos_t[:], cos_d[:])
            sin_t = consts.tile([128, S], F16)
            nc.sync.dma_start(sin_t[:], sin_d[:])
            for k in range(8):
                nc.sync.dma_start(xt_sb[:, k, HALF:S], xt_r[:, k, HALF:S])
            wv_sb = wpool.tile([128, 8, FG], F16)
            nc.sync.dma_start(wv_sb[:], wv_d[:].rearrange("(c p) f -> p c f", p=128))
            mask_t = consts.tile([128, 128], F16)
            nc.sync.dma_start(mask_t[:], mask_d[:])
            wo_sb = consts.tile([128, 2, DIM], F16)
            nc.sync.dma_start(wo_sb[:], wo_d[:].rearrange("(c p) d -> p c d", p=128))
            nc.vector.memset(vaug[:], 1.0)

            # ------- pools that coexist through the whole kernel -------
            # PSUM: stp 4 banks + obs 2 banks + aux 2 banks = 8
            stp = ctx.enter_context(tc.tile_pool(name="stps", bufs=2, space="PSUM"))
            obp = ctx.enter_context(tc.tile_pool(name="obp", bufs=1, space="PSUM"))
            aux = ctx.enter_context(tc.tile_pool(name="aux", bufs=2, space="PSUM"))
            rp = ctx.enter_context(tc.tile_pool(name="rope", bufs=8))
            ptp = ctx.enter_context(tc.tile_pool(name="ptp", bufs=20))
            sm = ctx.enter_context(tc.tile_pool(name="smp", bufs=4))
            bp = ctx.enter_context(tc.tile_pool(name="bcp", bufs=4))
            yp = ctx.enter_context(tc.tile_pool(name="ysb", bufs=4))

            # PE warm-up: ~4.5us of dummy matmuls on a zeroed tile so the
            # HAM clock-gate reaches 8/8 before the real projections issue
            # (PE idle >3.4us re-throttles to 1.2 GHz)
            wtile = consts.tile([128, 512], F16)
            nc.vector.memset(wtile[:], 0.0)
            wps = stp.tile([128, QTILE], F32, tag="st", name="warmps")
            for _ in range(20):
                nc.tensor.matmul(
                    wps[:, 0:512],
                    lhsT=wtile[:, 0:128],
                    rhs=wtile[:],
                    start=True,
                    stop=True,
                )

            # ---------- projection building blocks (aux quarter passes) ----
            _ctr = [0]

            def _nm(p):
                _ctr[0] += 1
                return f"{p}_{_ctr[0]}"

            def rope_half(dest, c, half, qc):
                """dest[:, c, half*HALF:...] = rope(qc) over one token half."""
                lo = half * HALF
                t1 = rp.tile([128, HALF], F16, tag="rope", name=_nm("t1"))
                nc.vector.tensor_mul(t1[:], qc[:], cos_t[:, lo:lo + HALF])
                rot = rp.tile([128, HALF], F16, tag="rope", name=_nm("rot"))
                for qq in range(4):
                    srcp = (qq ^ 1) * 32
                    nc.vector.tensor_copy(
                        rot[qq * 32:(qq + 1) * 32, :], qc[srcp:srcp + 32, :]
                    )
                t2 = rp.tile([128, HALF], F16, tag="rope", name=_nm("t2"))
                nc.vector.tensor_mul(t2[:], rot[:], sin_t[:, lo:lo + HALF])
                nc.vector.tensor_add(dest[:, c, lo:lo + HALF], t1[:], t2[:])

            def qk_half_items(dest, wsb, c, half, cast_on_act, key):
                """Trickle items (2 matmuls each) projecting one
                (dest, c-chunk, token-half); the last item ropes the half
                and carries `key` for emission-order need() forcing."""
                st = {}
                items = []
                lo = half * HALF

                def mk(tq, k0, fin):
                    def f(tq=tq, k0=k0, fin=fin):
                        if k0 == 0:
                            st[('ps', tq)] = aux.tile(
                                [128, 512], F32, tag="aux", name=_nm("psq")
                            )
                        ps = st[('ps', tq)]
                        for k in range(k0, k0 + 2):
                            nc.tensor.matmul(
                                ps[:],
                                lhsT=wsb[:, k, c * 128:(c + 1) * 128],
                                rhs=xt_sb[:, k, lo + tq * 512:lo + (tq + 1) * 512],
                                start=(k == 0),
                                stop=(k == 7),
                                skip_group_check=True,
                            )
                        if not fin:
                            return
                        if 'qc' not in st:
                            st['qc'] = rp.tile([128, HALF], F16, tag="rope",
                                               name=_nm("qc"))
                        if cast_on_act:
                            nc.scalar.copy(st['qc'][:, tq * 512:(tq + 1) * 512], ps[:])
                        else:
                            nc.vector.tensor_copy(
                                st['qc'][:, tq * 512:(tq + 1) * 512], ps[:]
                            )
                        if tq == 1:
                            rope_half(dest, c, half, st['qc'])
                    return f

                for tq in range(2):
                    for k0 in range(0, 8, 2):
                        fin = k0 == 6
                        items.append(
                            (key if (fin and tq == 1) else None, mk(tq, k0, fin))
                        )
                return items

            def v_items(tb):
                """Trickle items for V projection of token block tb."""
                st = {}

                def a(tb=tb):
                    st['ps'] = aux.tile([128, 512], F32, tag="aux", name=_nm("psv"))
                    for k in range(4):
                        nc.tensor.matmul(
                            st['ps'][:, 0:FG],
                            lhsT=xt_sb[:, k, tb * 128:(tb + 1) * 128],
                            rhs=wv_sb[:, k, :],
                            start=(k == 0),
                            stop=False,
                            skip_group_check=True,
                        )

                def b(tb=tb):
                    for k in range(4, 8):
                        nc.tensor.matmul(
                            st['ps'][:, 0:FG],
                            lhsT=xt_sb[:, k, tb * 128:(tb + 1) * 128],
                            rhs=wv_sb[:, k, :],
                            start=False,
                            stop=(k == 7),
                            skip_group_check=True,
                        )
                    nc.vector.tensor_copy(
                        vaug[:, tb, :].rearrange("p (h c) -> p h c", c=65)[:, :, 0:64],
                        st['ps'][:, 0:FG].rearrange("p (h d) -> p h d", d=64),
                    )

                return [(None, a), (('v', tb), b)]

            yt_r = yt_d[:].rearrange("(c p) t -> p c t", p=128)

            def out_items(th, dchunk):
                """Trickle items for one output-projection [128,1024] tile
                (two 512-col aux passes, one DMA)."""
                st = {}

                def mk(nn):
                    def f(nn=nn):
                        psy = aux.tile([128, 512], F32, tag="aux", name=_nm("psy"))
                        t0 = th * 1024 + nn * 512
                        for c2 in range(2):
                            nc.tensor.matmul(
                                psy[:],
                                lhsT=wo_sb[:, c2, dchunk * 128:(dchunk + 1) * 128],
                                rhs=zt_sb[:, c2, t0:t0 + 512],
                                start=(c2 == 0),
                                stop=(c2 == 1),
                                skip_group_check=True,
                            )
                        if 'yt' not in st:
                            st['yt'] = yp.tile([128, 1024], F16, tag="y",
                                               name=_nm("yt"))
                        nc.vector.tensor_copy(
                            st['yt'][:, nn * 512:(nn + 1) * 512], psy[:]
                        )
                        if nn == 1:
                            nc.sync.dma_start(
                                yt_r[:, dchunk, th * 1024:(th + 1) * 1024],
                                st['yt'][:],
                            )
                    return f

                return [(None, mk(0)), (None, mk(1))]

            trickle = deque()
            emitted = set()

            def pump(n):
                for _ in range(n):
                    if not trickle:
                        return
                    key, fn = trickle.popleft()
                    fn()
                    if key is not None:
                        emitted.add(key)

            def need(key):
                while key not in emitted:
                    assert trickle, f"need({key}) but trickle empty"
                    k, fn = trickle.popleft()
                    fn()
                    if k is not None:
                        emitted.add(k)

            # ---------------- prefix: pair-0 Q/K projections ----------------
            # (casts on ACT -- the exp stream hasn't started yet)
            for half in range(2):
                for _, it in qk_half_items(qt_sb, wq_sb, 0, half, True,
                                           ('q', 0, half)):
                    it()
                for _, it in qk_half_items(kt_sb, wk_sb, 0, half, True,
                                           ('k', 0, half)):
                    it()
            emitted.update({('q', 0, 0), ('k', 0, 0), ('q', 0, 1), ('k', 0, 1)})

            # trickle backlog: V (needed first), then pair-1 Q/K (+rope),
            # then (appended later) the tokens[0:1024) output projection
            for tb in range(NT):
                trickle.extend(v_items(tb))
            for half in range(2):
                trickle.extend(qk_half_items(qt_sb, wq_sb, 1, half, False,
                                             ('q', 1, half)))
                trickle.extend(qk_half_items(kt_sb, wk_sb, 1, half, False,
                                             ('k', 1, half)))

            # ---------------- attention units (head-serial) ----------------
            def unit(qt_i, pair, hh):
                qlo = qt_i * QTILE
                jmax = (qlo + QTILE) // 128
                base = 64 * hh
                hg = pair * 2 + hh
                pts = {}
                # emission-order producer guarantees (Tile deps only see
                # writes emitted BEFORE the reader)
                need(('q', pair, qt_i))
                for h in range(qt_i + 1):
                    need(('k', pair, h))
                obs = obp.tile([65, QTILE], F32, tag="o",
                               name=f"ob_{qt_i}_{pair}_{hh}")

                def emit_st(j):
                    qs = max(qlo, j * 128)
                    w = qlo + QTILE - qs
                    st = stp.tile([128, QTILE], F32, tag="st",
                                  name=f"st_{qt_i}_{pair}_{hh}_{j}")
                    for nn in range(0, w, 512):
                        ww = min(512, w - nn)
                        nc.tensor.matmul(
                            st[:, nn:nn + ww],
                            lhsT=kt_sb[base:base + 64, pair, j * 128:(j + 1) * 128],
                            rhs=qt_sb[base:base + 64, pair, qs + nn:qs + nn + ww],
                            start=True,
                            stop=True,
                        )
                    pt = ptp.tile([128, QTILE], F16, tag="pt", name=_nm("pt"))
                    nc.scalar.activation(pt[:, 0:w], st[:, 0:w], Exp, scale=0.125)
                    if j * 128 >= qlo:
                        nc.vector.tensor_mul(pt[:, 0:128], pt[:, 0:128], mask_t[:])
                    pts[j] = (pt, qs)

                def emit_pv(j):
                    need(('v', j))
                    pt, qs = pts.pop(j)
                    c0 = qs - qlo
                    while c0 < QTILE:
                        c1 = min((c0 // 512 + 1) * 512, QTILE)
                        last_j = (qlo + c1 - 1) // 128
                        nc.tensor.matmul(
                            obs[:, c0:c1],
                            lhsT=vaug[:, j, hg * 65:(hg + 1) * 65],
                            rhs=pt[:, c0 - (qs - qlo):c1 - (qs - qlo)],
                            start=(j == 0),
                            stop=(j == last_j),
                            skip_group_check=True,
                        )
                        c0 = c1

                emit_st(0)
                for j in range(jmax):
                    if j + 1 < jmax:
                        emit_st(j + 1)
                        pump(1)
                    emit_pv(j)
                # normalize: l is obs row 64 (the [V|1] ones column)
                lrow = sm.tile([1, QTILE], F32, tag="lrow",
                               name=f"lr_{qt_i}_{pair}_{hh}")
                nc.vector.tensor_copy(lrow[:], obs[64:65, :])
                rcp = sm.tile([1, QTILE], F32, tag="rcp",
                              name=f"rc_{qt_i}_{pair}_{hh}")
                nc.vector.reciprocal_approx_fast(rcp[:], lrow[:])
                bc = bp.tile([64, QTILE], F32, tag="bc",
                             name=f"bc_{qt_i}_{pair}_{hh}")
                nc.gpsimd.partition_broadcast(bc[:], rcp[:])
                nc.vector.tensor_mul(
                    zt_sb[hh * 64:(hh + 1) * 64, pair, qlo:qlo + QTILE],
                    obs[0:64, :],
                    bc[:],
                )
                pump(2)

            units = [(qt, pair, hh)
                     for pair in range(2) for qt in range(NQT) for hh in range(2)]
            for ui, (qt_i, pair, hh) in enumerate(units):
                unit(qt_i, pair, hh)
                if ui == 5:
                    # tokens [0,1024) of the output projection: zt for both
                    # pairs of qtile 0 is complete after the 6th unit
                    for dchunk in range(8):
                        trickle.extend(out_items(0, dchunk))

            # drain the trickle queue, then the second output-proj half
            pump(len(trickle))
            for dchunk in range(8):
                for _, it in out_items(1, dchunk):
                    it()

    nc.compile()
    return nc


def _host_inputs(x, Wq, Wk, Wv, Wo, S):
    """Per-core input maps (host-side sharding + layout prep)."""
    # de-interleave RoPE pairs within each head: (2i, 2i+1) -> (i, i+32)
    perm = np.concatenate([np.arange(0, HEAD_DIM, 2), np.arange(1, HEAD_DIM, 2)])
    rp = (np.arange(HPG)[:, None] * HEAD_DIM + perm[None, :]).reshape(-1)

    half = HEAD_DIM // 2
    inv_freq = THETA ** (-np.arange(half, dtype=np.float64) * 2.0 / HEAD_DIM)
    ang = np.arange(S, dtype=np.float64)[None, :] * inv_freq[:, None]  # [32, S]
    cos32 = np.cos(ang)
    sin32 = np.sin(ang)
    cos128 = np.tile(cos32, (4, 1)).astype(np.float16)
    sins128 = np.concatenate([-sin32, sin32, -sin32, sin32], axis=0).astype(np.float16)
    mask = (np.arange(128)[None, :] >= np.arange(128)[:, None]).astype(np.float16)

    in_maps = []
    for core in range(B * GROUPS):
        b, g = divmod(core, GROUPS)
        sl = slice(g * FG, (g + 1) * FG)
        in_maps.append(
            dict(
                xt=np.ascontiguousarray(x[b].T).astype(np.float16),
                wqt=np.ascontiguousarray(Wq[sl][rp].T).astype(np.float16),
                wkt=np.ascontiguousarray(Wk[sl][rp].T).astype(np.float16),
                wvt=np.ascontiguousarray(Wv[sl].T).astype(np.float16),
                wot=np.ascontiguousarray(Wo[:, sl].T).astype(np.float16),
                cos=cos128,
                sins=sins128,
                mask=mask,
            )
        )
    return in_maps


def _install_ntff_hook():
    """Provide antenv.axon_hooks if the image lacks it (NTFF profiling
    under axon; mirrors trn_agent_boot._ntff_profile_via_ctypes)."""
    try:
        from antenv.axon_hooks import get_axon_ntff_profile_hook  # noqa: F401
        return
    except ImportError:
        pass
    import contextlib
    import ctypes
    import types

    so_path = "/opt/axon/libaxon_pjrt.so"
    if not os.path.exists(so_path):
        return
    lib = ctypes.CDLL(so_path)
    if not hasattr(lib, "axon_start_nrt_profile"):
        return
    lib.axon_start_nrt_profile.argtypes = [
        ctypes.POINTER(ctypes.c_int64),
        ctypes.c_size_t,
    ]
    lib.axon_start_nrt_profile.restype = ctypes.c_int64
    lib.axon_stop_nrt_profile.argtypes = [ctypes.c_char_p]
    lib.axon_stop_nrt_profile.restype = ctypes.c_int64

    @contextlib.contextmanager
    def _hook(output_dir, device_ids):
        import jax

        jax.devices()
        if device_ids:
            ids = (ctypes.c_int64 * len(device_ids))(*device_ids)
            rc = lib.axon_start_nrt_profile(ids, len(device_ids))
        else:
            rc = lib.axon_start_nrt_profile(None, 0)
        if rc != 0:
            raise RuntimeError(f"axon_start_nrt_profile rc={rc}")
        try:
            yield
        finally:
            n = lib.axon_stop_nrt_profile(str(output_dir).encode())
            print(f"profile: {n} file(s) written to {output_dir}")

    mod = types.ModuleType("antenv.axon_hooks")
    _state = {"hook": _hook}
    mod.get_axon_ntff_profile_hook = lambda: _state["hook"]
    mod.set_axon_ntff_profile_hook = lambda h: _state.__setitem__("hook", h)
    import antenv

    antenv.axon_hooks = mod
    sys.modules["antenv.axon_hooks"] = mod


_NC_CACHE = {}


def _get_nc(S):
    if S not in _NC_CACHE:
        _NC_CACHE[S] = build_program(S)
    return _NC_CACHE[S]


def kernel(x, Wq, Wk, Wv, Wo, _trace=False, _tmpdir=None):
    x = np.asarray(x, dtype=np.float32)
    Wq = np.asarray(Wq, dtype=np.float32)
    Wk = np.asarray(Wk, dtype=np.float32)
    Wv = np.asarray(Wv, dtype=np.float32)
    Wo = np.asarray(Wo, dtype=np.float32)
    S = x.shape[1]

    if _trace:
        _install_ntff_hook()
    nc = _get_nc(S)
    in_maps = _host_inputs(x, Wq, Wk, Wv, Wo, S)
    res = run_bass_kernel_spmd(
        nc, in_maps, core_ids=list(range(8)), trace=_trace, tmpdir=_tmpdir
    )
    yts = [res.results[c]["yt"].astype(np.float32) for c in range(8)]
    y = np.stack(
        [sum(yts[b * GROUPS + g] for g in range(GROUPS)).T for b in range(B)]
    ).astype(np.float32)
    if _trace:
        kernel.last_results = res
    return y
